# revision 51
# baseline (speedup 1.0000x reference)
"""Trainium2 Bass kernel for FlattenIntraCycleMoELayer (top-2 MoE + general path).

Strategy (v4 — fully local, no cross-core communication):
  - Data-parallel over B (8 batteries per core); gen_W folded into each
    expert on host (gates sum to 1): out = x @ (g1*A_e1 + g2*A_e2),
    A_e = gen_W + expert_W[e], bias folded via an appended ones-row.
  - The gating network is REPLICATED per core for its own 8 batteries:
    the full W1 (17.3MB f16) is streamed through a 4-deep chunk pool
    and consumed by layer-1 matmuls as it lands.  This removes every
    cross-core dependency: the d_ff-sharded alternative needs a logits
    exchange, and both available transports cost ~55-80us of fixed
    latency (ncfw barrier handshake ~70us; SWDGE remote-DMA moves the
    payload as ~1000 4-byte packets plus ~30us of Q7 dispatch+trigger
    latency).  Streaming 17.3MB at ~350GB/s costs ~50us and overlaps
    the A/x loads, is deterministic, and is immune to launch skew.
  - Gating math: L1 accumulates into a [8, 4x512] PSUM (batteries on
    partitions), gelu + 16 PE transposes produce hT [128, 16, 8], L2 is
    16 tiny matmuls + a K=1 ones-row matmul folding gate_b2; top-2 via
    g2 = sigmoid(l2 - l1), g1 = 1 - g2 (equivalent to the reference's
    masked-softmax renorm up to the 1e-9 eps); one 16-col broadcast
    matmul replicates g1/g2 across partitions.
  - Combine is 2 fused ops per piece: t2 = g2*A_e2 (ACT scale-copy),
    wb = (g1*A_e1) + t2 (DVE scalar_tensor_tensor).
  - Main loop: kt-major matmuls per battery, combine one battery ahead,
    PSUM double-buffered, evictions batched 2 m-tiles per op (DVE/ACT).

Host-side prep only reshapes/pads/casts/re-parametrizes weights;
all model math runs on device.
"""

import numpy as np
import ml_dtypes


def _ensure_import_path():
    try:
        import concourse  # noqa: F401
    except ImportError:
        import sys
        for p in ("/opt/trn_rl_repo", "/root/.axon_site/_ro/trn_rl_repo"):
            if p not in sys.path:
                sys.path.insert(0, p)
        import concourse  # noqa: F401


_ensure_import_path()

import concourse.bass as bass  # noqa: E402
import concourse.tile as tile  # noqa: E402
from concourse import mybir  # noqa: E402
from concourse.bass import ds, ts  # noqa: E402
from concourse.alu_op_type import AluOpType  # noqa: E402
from concourse.tile import add_dep_helper  # noqa: E402

BF16 = mybir.dt.bfloat16
F32 = mybir.dt.float32
F16 = mybir.dt.float16
U32 = mybir.dt.uint32

# Problem shape constants (hardcoded per contest rules).
B, L, C, F = 64, 512, 3, 300
CF = C * F              # 900
K = CF + 1              # 901 contraction rows (data + ones row for bias)
KT = 8                  # k-tiles: 7 full + 1 remainder
KREM = K - 7 * 128      # 5 rows in the last k-tile
D = 512                 # d_model
E = 8                   # experts
NCORES = 8
BPC = B // NCORES       # 8 batteries per core
DLLM = 4096
GK = 4224               # padded gating contraction = 33*128
GKT = GK // 128         # 33
DFF = 2048
DFT = DFF // 128        # 16 d_ff tiles of 128
MT = L // 128           # 4 m-tiles per battery
WCH = 2                 # w1 k-tiles per streamed chunk
NCH = 17                # 16 chunks of 2 k-tiles + 1 of 1 = 33
NWARM = 16              # PE warm-up junk matmuls before the main loop


def build_program(nc):
    from contextlib import ExitStack

    xmain = nc.dram_tensor("xmain", [BPC, 128, 7, L], BF16, kind="ExternalInput")
    xrem = nc.dram_tensor("xrem", [BPC, KREM, L], BF16, kind="ExternalInput")
    amain = nc.dram_tensor("amain", [128, 7, E, D], BF16, kind="ExternalInput")
    arem = nc.dram_tensor("arem", [KREM, E, D], BF16, kind="ExternalInput")
    ginp = nc.dram_tensor("ginp", [128, GKT * BPC], F16, kind="ExternalInput")
    w1d = nc.dram_tensor("w1f", [128, GKT * DFF], F16, kind="ExternalInput")
    w2p = nc.dram_tensor("w2f", [128, DFT * E], F32, kind="ExternalInput")
    b2d = nc.dram_tensor("b2row", [1, E], F32, kind="ExternalInput")
    id64d = nc.dram_tensor("id64", [B, B], F32, kind="ExternalInput")
    outd = nc.dram_tensor("out", [BPC, 128, MT, D], BF16, kind="ExternalOutput")

    with tile.TileContext(nc) as tc, ExitStack() as ctx:
        singles = ctx.enter_context(tc.tile_pool(name="singles", bufs=1))
        gpool = ctx.enter_context(tc.tile_pool(name="gate", bufs=1))

        w1_ctx = ExitStack()
        w1pool = w1_ctx.enter_context(tc.tile_pool(name="w1s", bufs=6))
        gps1_ctx = ExitStack()
        gps1 = gps1_ctx.enter_context(tc.tile_pool(name="gpsum1", bufs=1,
                                                   space="PSUM"))

        # ---------- DMAs -----------------------------------------------
        # w1 chunks alternate between the two HWDGE rings (sync/scalar);
        # the 4-deep pool keeps issues ahead of consumption.  A and x are
        # interleaved behind the w1 stream on both rings.
        ginT_sb = gpool.tile([128, GKT, BPC], F16)
        nc.sync.dma_start(out=ginT_sb.rearrange("p k b -> p (k b)"), in_=ginp.ap())
        w2_sb = gpool.tile([128, DFT, E], F32)
        nc.sync.dma_start(out=w2_sb.rearrange("p j e -> p (j e)"), in_=w2p.ap())
        b2_sb = gpool.tile([1, E], F32)
        nc.sync.dma_start(out=b2_sb, in_=b2d.ap())
        id64 = gpool.tile([B, B], F32)
        nc.sync.dma_start(out=id64, in_=id64d.ap())

        w1_ap = w1d.ap().rearrange("p (k f) -> p k f", k=GKT)
        w1c_tiles = []
        for ci in range(NCH):
            lo = ci * WCH
            hi = min(lo + WCH, GKT)
            eng = nc.sync if ci % 2 == 0 else nc.scalar
            w1c = w1pool.tile([128, WCH, DFF], F16, tag="w1c")
            eng.dma_start(
                out=w1c[:, 0:hi - lo, :].rearrange("p k f -> p (k f)"),
                in_=w1_ap[:, lo:hi, :].rearrange("p k f -> p (k f)"),
            )
            w1c_tiles.append(w1c)

        # x battery 0 early on sync (needed at main-loop start).
        xmain_ap = xmain.ap()
        xrem_ap = xrem.ap()
        xb_tiles = [None] * BPC

        def load_xb(b, eng):
            xb = xb_tiles[b]
            eng.dma_start(
                out=xb[:, 0:7, :].rearrange("p k l -> p (k l)"),
                in_=xmain_ap[b].rearrange("p k l -> p (k l)"),
            )
            eng.dma_start(out=xb[0:KREM, 7, :], in_=xrem_ap[b])

        for b in range(BPC):
            xb = singles.tile([128, KT, L], BF16, tag=f"xb{b}")
            xb_tiles[b] = xb
        load_xb(0, nc.sync)

        # A: even k-tiles on sync, odd + remainder on scalar; late x
        # batteries split across both rings behind A.
        A_sb = singles.tile([128, E, KT, D], BF16)
        am_ap = amain.ap()
        nc.vector.memset(A_sb[:, :, 7, :], 0.0)
        for kt in (1, 3, 5):
            nc.scalar.dma_start(out=A_sb[:, :, kt, :], in_=am_ap[:, kt, :, :])
        nc.scalar.dma_start(out=A_sb[0:KREM, :, 7, :], in_=arem.ap())
        for kt in (0, 2, 4, 6):
            nc.sync.dma_start(out=A_sb[:, :, kt, :], in_=am_ap[:, kt, :, :])
        for b in (1, 2, 3):
            load_xb(b, nc.scalar)
        for b in (4, 5):
            load_xb(b, nc.sync)
        for b in (6, 7):
            load_xb(b, nc.scalar)

        # ---------- DVE constants (no deps, ~1us) ----------------------
        ones_sb = gpool.tile([BPC, 128], F32)
        nc.vector.memset(ones_sb, 1.0)
        jt = gpool.tile([128, 512], F16, tag="junk")
        nc.vector.memset(jt, 0.0)
        # preload the Tanh/Sigmoid ACT tables off the critical path (the
        # engine otherwise pays a ~1.3us table load at first use).
        warm_act = gpool.tile([1, 8], F32, tag="wact")
        nc.scalar.activation(out=warm_act, in_=jt[0:1, 0:8],
                             func=mybir.ActivationFunctionType.Tanh,
                             scale=1.0)
        nc.scalar.activation(out=warm_act, in_=jt[0:1, 0:8],
                             func=mybir.ActivationFunctionType.Sigmoid,
                             scale=1.0)

        # ---------- gating layer 1 (PE, chunk-streamed) ----------------
        # psum_h[b, nb*512 + f] accumulates over all 33 k-tiles.  Junk
        # matmuls into a scratch PSUM bank fill the PE between chunk
        # arrivals so the HAM clock-gate stays at full speed (the real
        # L1 work is DMA-paced; without the filler the PE idles several
        # us per chunk and gets demoted to half clock).
        psum_h = gps1.tile([BPC, 4, 512], F32, bufs=1)
        jps = gps1.tile([BPC, 512], F32, bufs=1)
        for ci in range(NCH):
            for kl in range(min(WCH, GKT - ci * WCH)):
                gkt = ci * WCH + kl
                for nb in range(4):
                    nc.tensor.matmul(
                        out=psum_h[:, nb, :],
                        lhsT=ginT_sb[:, gkt, :],
                        rhs=w1c_tiles[ci][:, kl, nb * 512:(nb + 1) * 512],
                        start=(gkt == 0), stop=(gkt == GKT - 1),
                    )
            for j in range(3):
                nc.tensor.matmul(out=jps, lhsT=jt[:, 0:BPC], rhs=jt,
                                 start=True, stop=True)

        # transpose the pre-activation first (gelu is pointwise and
        # commutes with the transpose): hX [128, 16, 8] keeps the gelu
        # intermediates at 512B/partition instead of 8KB.  The h8
        # eviction is split per PSUM quarter so transposes start early,
        # and psum_h is freed before the transpose PSUM tiles (4 bufs)
        # are allocated.
        h8 = gpool.tile([BPC, DFF], F32)
        hev0 = nc.vector.tensor_copy(out=h8[:, 0:512], in_=psum_h[:, 0, :])
        nc.scalar.activation(out=h8[:, 512:1024], in_=psum_h[:, 1, :],
                             func=mybir.ActivationFunctionType.Copy)
        nc.vector.tensor_copy(out=h8[:, 1024:1536], in_=psum_h[:, 2, :])
        nc.scalar.activation(out=h8[:, 1536:2048], in_=psum_h[:, 3, :],
                             func=mybir.ActivationFunctionType.Copy)
        gps1_ctx.close()
        gps_ctx = ExitStack()
        gps = gps_ctx.enter_context(tc.tile_pool(name="gpsum2", bufs=1,
                                                 space="PSUM"))
        hX = gpool.tile([128, DFT, BPC], F32)
        for j in range(DFT):
            pst = gps.tile([128, BPC], F32, bufs=4, tag="pst")
            nc.tensor.transpose(
                out=pst, in_=h8[:, j * 128:(j + 1) * 128],
                identity=id64[0:BPC, 0:BPC],
            )
            if j % 2 == 0:
                nc.vector.tensor_copy(out=hX[:, j, :], in_=pst)
            else:
                nc.scalar.activation(out=hX[:, j, :], in_=pst,
                                     func=mybir.ActivationFunctionType.Copy)

        # gelu (tanh approx) on the transposed layout:
        #   h = (0.5*x) * (1 + tanh(0.79788456*(x + 0.044715*x^3)))
        hXf = hX.rearrange("p j b -> p (j b)")
        g_x2 = gpool.tile([128, DFT * BPC], F32)
        nc.vector.tensor_tensor(out=g_x2, in0=hXf, in1=hXf, op=AluOpType.mult)
        g_xh = gpool.tile([128, DFT * BPC], F32)
        nc.vector.tensor_scalar_mul(g_xh, hXf, 0.5)
        g_p = gpool.tile([128, DFT * BPC], F32)
        nc.vector.tensor_scalar(g_p, g_x2, 0.044715, 1.0,
                                AluOpType.mult, AluOpType.add)
        g_u = gpool.tile([128, DFT * BPC], F32)
        nc.vector.tensor_tensor(out=g_u, in0=hXf, in1=g_p, op=AluOpType.mult)
        g_t = gpool.tile([128, DFT * BPC], F32)
        nc.scalar.activation(out=g_t, in_=g_u,
                             func=mybir.ActivationFunctionType.Tanh,
                             scale=0.7978845608028654)
        hT_sb = gpool.tile([128, DFT, BPC], F32)
        nc.vector.scalar_tensor_tensor(out=hT_sb.rearrange("p j b -> p (j b)"),
                                       in0=g_t, scalar=1.0, in1=g_xh,
                                       op0=AluOpType.add, op1=AluOpType.mult)

        # layer 2 (+ K=1 ones-row matmul folding gate_b2)
        psum_l = gps.tile([BPC, E], F32, bufs=2, tag="psl")
        for j in range(DFT):
            nc.tensor.matmul(out=psum_l, lhsT=hT_sb[:, j, :], rhs=w2_sb[:, j, :],
                             start=(j == 0), stop=False)
        nc.tensor.matmul(out=psum_l, lhsT=ones_sb[0:1, 0:BPC], rhs=b2_sb,
                         start=False, stop=True)
        logits_my = gpool.tile([BPC, E], F32)
        lev = nc.vector.tensor_copy(out=logits_my, in_=psum_l)

        # top-2 gates: g2 = sigmoid(l2 - l1), g1 = 1 - g2
        sorted8 = gpool.tile([BPC, E], F32)
        sidx = gpool.tile([BPC, E], U32)
        nc.vector.max(out=sorted8, in_=logits_my)
        nc.vector.max_index(out=sidx, in_max=sorted8, in_values=logits_my)
        diff = gpool.tile([BPC, 1], F32)
        nc.vector.tensor_tensor(out=diff, in0=sorted8[:, 1:2],
                                in1=sorted8[:, 0:1], op=AluOpType.subtract)
        g2c = gpool.tile([BPC, 1], F32)
        nc.scalar.activation(out=g2c, in_=diff,
                             func=mybir.ActivationFunctionType.Sigmoid,
                             scale=1.0)
        g1c = gpool.tile([BPC, 1], F32)
        nc.vector.tensor_scalar(g1c, g2c, -1.0, 1.0,
                                AluOpType.mult, AluOpType.add)

        # broadcast g1/g2 of each battery to all 128 partitions with one
        # matmul: rhs = [diag(g1) | diag(g2)] (8 x 16), lhsT = ones (8 x 128).
        rhs8 = gpool.tile([BPC, 2, BPC], F32)
        nc.vector.tensor_scalar_mul(rhs8[:, 0, :], id64[0:BPC, 0:BPC], g1c)
        nc.scalar.activation(out=rhs8[:, 1, :], in_=id64[0:BPC, 0:BPC],
                             func=mybir.ActivationFunctionType.Copy,
                             scale=g2c)
        psum_bc = gps.tile([128, 2 * BPC], F32, bufs=2, tag="pbc")
        nc.tensor.matmul(out=psum_bc, lhsT=ones_sb,
                         rhs=rhs8.rearrange("p s b -> p (s b)"),
                         start=True, stop=True)
        bcA = gpool.tile([128, 2, BPC], F32)
        nc.vector.tensor_copy(out=bcA.rearrange("p s b -> p (s b)"), in_=psum_bc)

        # PE warm-up gated on the first h8 eviction: keeps the HAM
        # clock-gate at full speed through the transpose/L2/top-2 chain
        # and into the fused main-loop matmuls.
        jps2 = gps.tile([BPC, 512], F32, bufs=2, tag="pbc")
        for j in range(NWARM):
            jmm = nc.tensor.matmul(
                out=jps2, lhsT=jt[:, 0:BPC], rhs=jt,
                start=True, stop=True,
            )
            if j == 0:
                add_dep_helper(jmm.ins, hev0.ins, sync=True,
                               reason="warm-up spans the post-L1 chain")

        gps_ctx.close()
        w1_ctx.close()

        # ---------- main fused phase -----------------------------------
        mps = ctx.enter_context(tc.tile_pool(name="mpsum", bufs=2, space="PSUM"))
        wbpool = ctx.enter_context(tc.tile_pool(name="wbs", bufs=2))
        scpool = ctx.enter_context(tc.tile_pool(name="scratch", bufs=2))
        opool = ctx.enter_context(tc.tile_pool(name="outs", bufs=3))

        def _vload(eng, ap, name):
            reg = eng.alloc_register(name)
            eng.reg_load(reg, ap)
            val = eng.snap(reg, donate=True)
            return nc.s_assert_within(val, 0, E - 1, skip_runtime_assert=True)

        def combine(b, pieces=2):
            """wb = g1*A_e1 + g2*A_e2 for battery b, 2 fused ops per piece."""
            rv1 = _vload(nc.vector, sidx[b:b + 1, 0:1], f"e1_{b}")
            rv2 = _vload(nc.scalar, sidx[b:b + 1, 1:2], f"e2_{b}")
            wb = wbpool.tile([128, KT, D], BF16)
            w = KT // pieces
            for h in range(pieces):
                kts = slice(h * w, (h + 1) * w)
                t2 = scpool.tile([128, w, D], BF16, tag=f"t2_{pieces}")
                nc.scalar.activation(
                    out=t2.rearrange("p k d -> p (k d)"),
                    in_=A_sb[:, ds(rv2, 1), kts, :].rearrange("p o k d -> p (o k d)"),
                    func=mybir.ActivationFunctionType.Copy,
                    scale=bcA[:, 1, b:b + 1],
                )
                nc.vector.scalar_tensor_tensor(
                    out=wb[:, kts, :].rearrange("p k d -> p (k d)"),
                    in0=A_sb[:, ds(rv1, 1), kts, :].rearrange("p o k d -> p (o k d)"),
                    scalar=bcA[:, 0, b:b + 1],
                    in1=t2.rearrange("p k d -> p (k d)"),
                    op0=AluOpType.mult, op1=AluOpType.add,
                )
            return wb

        def battery(b, wb):
            xb = xb_tiles[b]
            pm = mps.tile([128, MT, D], F32, tag="mp")
            for kt in range(KT):
                np_ = KREM if kt == 7 else 128
                for m in range(MT):
                    nc.tensor.matmul(
                        out=pm[:, m, :],
                        lhsT=xb[0:np_, kt, ts(m, 128)],
                        rhs=wb[0:np_, kt, :],
                        start=(kt == 0), stop=(kt == KT - 1),
                    )
            osb = opool.tile([128, MT, D], BF16, tag="osb")
            nc.vector.tensor_copy(
                out=osb[:, 0:2, :].rearrange("p m d -> p (m d)"),
                in_=pm[:, 0:2, :].rearrange("p m d -> p (m d)"),
            )
            nc.scalar.activation(
                out=osb[:, 2:4, :].rearrange("p m d -> p (m d)"),
                in_=pm[:, 2:4, :].rearrange("p m d -> p (m d)"),
                func=mybir.ActivationFunctionType.Copy,
            )
            return nc.sync.dma_start(
                out=outd.ap()[b].rearrange("p m d -> p (m d)"),
                in_=osb.rearrange("p m d -> p (m d)"),
            )

        wbs = {0: combine(0, pieces=4), 1: combine(1, pieces=4)}
        for b in range(BPC):
            battery(b, wbs.pop(b))
            if b + 2 < BPC:
                wbs[b + 2] = combine(b + 2)


def make_nc():
    from concourse import bacc
    nc = bacc.Bacc("TRN2", target_bir_lowering=False, debug=False,
                   num_devices=NCORES)
    build_program(nc)
    nc.finalize()
    return nc


def prep_inputs(cycle_curve_data, cycle_numbers, DKP_embeddings,
                gate_W1, gate_b1, gate_W2, gate_b2,
                expert_W, expert_b, gen_W, gen_b):
    """Host-side layout prep (reshape/pad/cast/weight-fold). Returns in_maps."""
    f32 = np.float32
    bf16 = ml_dtypes.bfloat16

    # fused expert weights A_e = gen_W + expert_W[e]; ones-row bias.
    A = np.empty((E, K, D), dtype=f32)
    A[:, :CF, :] = np.asarray(expert_W, dtype=f32) + np.asarray(gen_W, dtype=f32)
    A[:, CF, :] = np.asarray(expert_b, dtype=f32) + np.asarray(gen_b, dtype=f32)
    Abf = A.astype(bf16)
    # [128, 7(kt), E, D] so each k-tile is one contiguous DMA chunk.
    amain = np.ascontiguousarray(
        Abf[:, :896, :].reshape(E, 7, 128, D).transpose(2, 1, 0, 3))
    arem = np.ascontiguousarray(Abf[:, 896:K, :].transpose(1, 0, 2))

    # x transposed with ones-row, partition-major.
    x = np.asarray(cycle_curve_data, dtype=f32).reshape(B, L, CF)
    xT = np.empty((B, K, L), dtype=bf16)
    xT[:, :CF, :] = x.transpose(0, 2, 1).astype(bf16)
    xT[:, CF, :] = np.asarray(1.0, dtype=bf16)
    xmain = np.ascontiguousarray(
        xT[:, :896, :].reshape(B, 7, 128, L).transpose(0, 2, 1, 3))
    xrem = np.ascontiguousarray(xT[:, 896:K, :])

    # gating input, partition-major, per-core battery slice.
    g = np.zeros((GK, B), dtype=f32)
    g[:DLLM, :] = np.asarray(DKP_embeddings, dtype=f32).T
    g[DLLM, :] = np.asarray(cycle_numbers, dtype=f32)[:, 0]
    g[DLLM + 1, :] = 1.0
    gpm = g.reshape(GKT, 128, B).transpose(1, 0, 2).astype(np.float16)

    # full W1 (replicated to every core), partition-major.
    W1p = np.zeros((GK, DFF), dtype=f32)
    W1p[:DLLM + 1, :] = np.asarray(gate_W1, dtype=f32)
    W1p[DLLM + 1, :] = np.asarray(gate_b1, dtype=f32)
    w1f = np.ascontiguousarray(
        W1p.reshape(GKT, 128, DFF).transpose(1, 0, 2)
        .reshape(128, GKT * DFF).astype(np.float16))

    w2f = np.ascontiguousarray(
        np.asarray(gate_W2, dtype=f32).reshape(DFT, 128, E)
        .transpose(1, 0, 2).reshape(128, DFT * E))
    b2row = np.ascontiguousarray(np.asarray(gate_b2, dtype=f32).reshape(1, E))
    id64 = np.eye(B, dtype=f32)

    in_maps = []
    for c in range(NCORES):
        ginp = np.ascontiguousarray(
            gpm[:, :, c * BPC:(c + 1) * BPC].reshape(128, GKT * BPC))
        in_maps.append({
            "xmain": np.ascontiguousarray(xmain[c * BPC:(c + 1) * BPC]),
            "xrem": np.ascontiguousarray(xrem[c * BPC:(c + 1) * BPC]),
            "amain": amain,
            "arem": arem,
            "ginp": ginp,
            "w1f": w1f,
            "w2f": w2f,
            "b2row": b2row,
            "id64": id64,
        })
    return in_maps


_CACHED = {}


def run(inputs, trace=False, tmpdir=None):
    """Run on the 8 NeuronCores; returns (full_output, BassKernelResults)."""
    from concourse import bass_utils
    in_maps = prep_inputs(**inputs)
    nc = _CACHED.get("nc")
    if nc is None:
        nc = make_nc()
        _CACHED["nc"] = nc
    res = bass_utils.run_bass_kernel_spmd(
        nc, in_maps, core_ids=list(range(NCORES)), trace=trace, tmpdir=tmpdir
    )
    outs = [np.asarray(r["out"]) for r in res.results]
    full = np.concatenate(outs, axis=0)          # [B, 128, MT, D] bf16
    full = full.transpose(0, 2, 1, 3).reshape(B, L, D).astype(np.float32)
    return full, res


def kernel(**inputs):
    full, _ = run(inputs, trace=False)
    return full


# revision 52
# speedup vs baseline: 1.0810x; 1.0810x over previous
"""Trainium2 Bass kernel for FlattenIntraCycleMoELayer (top-2 MoE + general path).

Strategy (v4 — fully local, no cross-core communication):
  - Data-parallel over B (8 batteries per core); gen_W folded into each
    expert on host (gates sum to 1): out = x @ (g1*A_e1 + g2*A_e2),
    A_e = gen_W + expert_W[e], bias folded via an appended ones-row.
  - The gating network is REPLICATED per core for its own 8 batteries:
    the full W1 (17.3MB f16) is streamed through a 4-deep chunk pool
    and consumed by layer-1 matmuls as it lands.  This removes every
    cross-core dependency: the d_ff-sharded alternative needs a logits
    exchange, and both available transports cost ~55-80us of fixed
    latency (ncfw barrier handshake ~70us; SWDGE remote-DMA moves the
    payload as ~1000 4-byte packets plus ~30us of Q7 dispatch+trigger
    latency).  Streaming 17.3MB at ~350GB/s costs ~50us and overlaps
    the A/x loads, is deterministic, and is immune to launch skew.
  - Gating math: L1 accumulates into a [8, 4x512] PSUM (batteries on
    partitions), gelu + 16 PE transposes produce hT [128, 16, 8], L2 is
    16 tiny matmuls + a K=1 ones-row matmul folding gate_b2; top-2 via
    g2 = sigmoid(l2 - l1), g1 = 1 - g2 (equivalent to the reference's
    masked-softmax renorm up to the 1e-9 eps); one 16-col broadcast
    matmul replicates g1/g2 across partitions.
  - Combine is 2 fused ops per piece: t2 = g2*A_e2 (ACT scale-copy),
    wb = (g1*A_e1) + t2 (DVE scalar_tensor_tensor).
  - Main loop: kt-major matmuls per battery, combine one battery ahead,
    PSUM double-buffered, evictions batched 2 m-tiles per op (DVE/ACT).

Host-side prep only reshapes/pads/casts/re-parametrizes weights;
all model math runs on device.
"""

import numpy as np
import ml_dtypes


def _ensure_import_path():
    try:
        import concourse  # noqa: F401
    except ImportError:
        import sys
        for p in ("/opt/trn_rl_repo", "/root/.axon_site/_ro/trn_rl_repo"):
            if p not in sys.path:
                sys.path.insert(0, p)
        import concourse  # noqa: F401


_ensure_import_path()

import concourse.bass as bass  # noqa: E402
import concourse.tile as tile  # noqa: E402
from concourse import mybir  # noqa: E402
from concourse.bass import ds, ts  # noqa: E402
from concourse.alu_op_type import AluOpType  # noqa: E402
from concourse.tile import add_dep_helper  # noqa: E402

BF16 = mybir.dt.bfloat16
F32 = mybir.dt.float32
F16 = mybir.dt.float16
U32 = mybir.dt.uint32

# Problem shape constants (hardcoded per contest rules).
B, L, C, F = 64, 512, 3, 300
CF = C * F              # 900
K = CF + 1              # 901 contraction rows (data + ones row for bias)
KT = 8                  # k-tiles: 7 full + 1 remainder
KREM = K - 7 * 128      # 5 rows in the last k-tile
D = 512                 # d_model
E = 8                   # experts
NCORES = 8
BPC = B // NCORES       # 8 batteries per core
DLLM = 4096
GK = 4224               # padded gating contraction = 33*128
GKT = GK // 128         # 33
DFF = 2048
DFT = DFF // 128        # 16 d_ff tiles of 128
MT = L // 128           # 4 m-tiles per battery
WCH = 2                 # w1 k-tiles per streamed chunk
NCH = 17                # 16 chunks of 2 k-tiles + 1 of 1 = 33
NWARM = 16              # PE warm-up junk matmuls before the main loop


def build_program(nc):
    from contextlib import ExitStack

    xmain = nc.dram_tensor("xmain", [BPC, 128, 7, L], BF16, kind="ExternalInput")
    xrem = nc.dram_tensor("xrem", [BPC, KREM, L], BF16, kind="ExternalInput")
    amain = nc.dram_tensor("amain", [128, 7, E, D], BF16, kind="ExternalInput")
    arem = nc.dram_tensor("arem", [KREM, E, D], BF16, kind="ExternalInput")
    ginp = nc.dram_tensor("ginp", [128, GKT * BPC], F16, kind="ExternalInput")
    w1d = nc.dram_tensor("w1f", [128, GKT * DFF], F16, kind="ExternalInput")
    w2p = nc.dram_tensor("w2f", [128, DFT * E], F32, kind="ExternalInput")
    b2d = nc.dram_tensor("b2row", [1, E], F32, kind="ExternalInput")
    id64d = nc.dram_tensor("id64", [B, B], F32, kind="ExternalInput")
    outd = nc.dram_tensor("out", [BPC, 128, MT, D], BF16, kind="ExternalOutput")

    with tile.TileContext(nc) as tc, ExitStack() as ctx:
        singles = ctx.enter_context(tc.tile_pool(name="singles", bufs=1))
        gpool = ctx.enter_context(tc.tile_pool(name="gate", bufs=1))

        w1_ctx = ExitStack()
        w1pool = w1_ctx.enter_context(tc.tile_pool(name="w1s", bufs=6))
        gps1_ctx = ExitStack()
        gps1 = gps1_ctx.enter_context(tc.tile_pool(name="gpsum1", bufs=1,
                                                   space="PSUM"))

        # ---------- DMAs -----------------------------------------------
        # w1 chunks alternate between the two HWDGE rings (sync/scalar);
        # the 4-deep pool keeps issues ahead of consumption.  A and x are
        # interleaved behind the w1 stream on both rings.
        ginT_sb = gpool.tile([128, GKT, BPC], F16)
        nc.sync.dma_start(out=ginT_sb.rearrange("p k b -> p (k b)"), in_=ginp.ap())
        w2_sb = gpool.tile([128, DFT, E], F32)
        nc.sync.dma_start(out=w2_sb.rearrange("p j e -> p (j e)"), in_=w2p.ap())
        b2_sb = gpool.tile([1, E], F32)
        nc.sync.dma_start(out=b2_sb, in_=b2d.ap())
        id64 = gpool.tile([B, B], F32)
        nc.sync.dma_start(out=id64, in_=id64d.ap())

        w1_ap = w1d.ap().rearrange("p (k f) -> p k f", k=GKT)
        w1c_tiles = []
        for ci in range(NCH):
            lo = ci * WCH
            hi = min(lo + WCH, GKT)
            eng = nc.sync if ci % 2 == 0 else nc.scalar
            w1c = w1pool.tile([128, WCH, DFF], F16, tag="w1c")
            eng.dma_start(
                out=w1c[:, 0:hi - lo, :].rearrange("p k f -> p (k f)"),
                in_=w1_ap[:, lo:hi, :].rearrange("p k f -> p (k f)"),
            )
            w1c_tiles.append(w1c)

        # x battery 0 early on sync (needed at main-loop start).
        xmain_ap = xmain.ap()
        xrem_ap = xrem.ap()
        xb_tiles = [None] * BPC

        def load_xb(b, eng):
            xb = xb_tiles[b]
            eng.dma_start(
                out=xb[:, 0:7, :].rearrange("p k l -> p (k l)"),
                in_=xmain_ap[b].rearrange("p k l -> p (k l)"),
            )
            eng.dma_start(out=xb[0:KREM, 7, :], in_=xrem_ap[b])

        for b in range(BPC):
            xb = singles.tile([128, KT, L], BF16, tag=f"xb{b}")
            xb_tiles[b] = xb
        load_xb(0, nc.sync)

        # All scalar-ring issues sit at the TOP of the ACT stream: a
        # dma_start stalls its issuing engine when ring credits run out,
        # so nothing DMA-related may be emitted after the gating ACT ops
        # (the h8 evictions/tanh would queue behind the stall).  The
        # sync engine is compute-free, so its issues can block freely.
        A_sb = singles.tile([128, E, KT, D], BF16)
        am_ap = amain.ap()
        nc.vector.memset(A_sb[:, :, 7, :], 0.0)
        for kt in (1, 3, 5):
            nc.scalar.dma_start(out=A_sb[:, :, kt, :], in_=am_ap[:, kt, :, :])
        for b in (1, 2, 3, 4):
            load_xb(b, nc.scalar)
        for kt in (0, 2, 4, 6):
            nc.sync.dma_start(out=A_sb[:, :, kt, :], in_=am_ap[:, kt, :, :])
        nc.sync.dma_start(out=A_sb[0:KREM, :, 7, :], in_=arem.ap())
        for b in (5, 6, 7):
            load_xb(b, nc.sync)

        # ---------- DVE constants (no deps, ~1us) ----------------------
        ones_sb = gpool.tile([BPC, 128], F32)
        nc.vector.memset(ones_sb, 1.0)
        jt = gpool.tile([128, 512], F16, tag="junk")
        nc.vector.memset(jt, 0.0)
        # preload the Tanh/Sigmoid ACT tables off the critical path (the
        # engine otherwise pays a ~1.3us table load at first use).
        warm_act = gpool.tile([1, 8], F32, tag="wact")
        nc.scalar.activation(out=warm_act, in_=jt[0:1, 0:8],
                             func=mybir.ActivationFunctionType.Tanh,
                             scale=1.0)
        nc.scalar.activation(out=warm_act, in_=jt[0:1, 0:8],
                             func=mybir.ActivationFunctionType.Sigmoid,
                             scale=1.0)

        # ---------- gating layer 1 (PE, chunk-streamed) ----------------
        # psum_h[b, nb*512 + f] accumulates over all 33 k-tiles.  Junk
        # matmuls into a scratch PSUM bank fill the PE between chunk
        # arrivals so the HAM clock-gate stays at full speed (the real
        # L1 work is DMA-paced; without the filler the PE idles several
        # us per chunk and gets demoted to half clock).
        psum_h = gps1.tile([BPC, 4, 512], F32, bufs=1)
        jps = gps1.tile([BPC, 512], F32, bufs=1)
        for ci in range(NCH):
            for kl in range(min(WCH, GKT - ci * WCH)):
                gkt = ci * WCH + kl
                for nb in range(4):
                    nc.tensor.matmul(
                        out=psum_h[:, nb, :],
                        lhsT=ginT_sb[:, gkt, :],
                        rhs=w1c_tiles[ci][:, kl, nb * 512:(nb + 1) * 512],
                        start=(gkt == 0), stop=(gkt == GKT - 1),
                    )
            for j in range(3):
                nc.tensor.matmul(out=jps, lhsT=jt[:, 0:BPC], rhs=jt,
                                 start=True, stop=True)

        # transpose the pre-activation first (gelu is pointwise and
        # commutes with the transpose): hX [128, 16, 8] keeps the gelu
        # intermediates at 512B/partition instead of 8KB.  The h8
        # eviction is split per PSUM quarter so transposes start early,
        # and psum_h is freed before the transpose PSUM tiles (4 bufs)
        # are allocated.
        h8 = gpool.tile([BPC, DFF], F32)
        hev0 = nc.vector.tensor_copy(out=h8[:, 0:512], in_=psum_h[:, 0, :])
        nc.scalar.activation(out=h8[:, 512:1024], in_=psum_h[:, 1, :],
                             func=mybir.ActivationFunctionType.Copy)
        nc.vector.tensor_copy(out=h8[:, 1024:1536], in_=psum_h[:, 2, :])
        nc.scalar.activation(out=h8[:, 1536:2048], in_=psum_h[:, 3, :],
                             func=mybir.ActivationFunctionType.Copy)
        gps1_ctx.close()
        gps_ctx = ExitStack()
        gps = gps_ctx.enter_context(tc.tile_pool(name="gpsum2", bufs=1,
                                                 space="PSUM"))
        hX = gpool.tile([128, DFT, BPC], F32)
        for j in range(DFT):
            pst = gps.tile([128, BPC], F32, bufs=4, tag="pst")
            nc.tensor.transpose(
                out=pst, in_=h8[:, j * 128:(j + 1) * 128],
                identity=id64[0:BPC, 0:BPC],
            )
            if j % 2 == 0:
                nc.vector.tensor_copy(out=hX[:, j, :], in_=pst)
            else:
                nc.scalar.activation(out=hX[:, j, :], in_=pst,
                                     func=mybir.ActivationFunctionType.Copy)

        # gelu (tanh approx) on the transposed layout:
        #   h = (0.5*x) * (1 + tanh(0.79788456*(x + 0.044715*x^3)))
        hXf = hX.rearrange("p j b -> p (j b)")
        g_x2 = gpool.tile([128, DFT * BPC], F32)
        nc.vector.tensor_tensor(out=g_x2, in0=hXf, in1=hXf, op=AluOpType.mult)
        g_xh = gpool.tile([128, DFT * BPC], F32)
        nc.vector.tensor_scalar_mul(g_xh, hXf, 0.5)
        g_p = gpool.tile([128, DFT * BPC], F32)
        nc.vector.tensor_scalar(g_p, g_x2, 0.044715, 1.0,
                                AluOpType.mult, AluOpType.add)
        g_u = gpool.tile([128, DFT * BPC], F32)
        nc.vector.tensor_tensor(out=g_u, in0=hXf, in1=g_p, op=AluOpType.mult)
        g_t = gpool.tile([128, DFT * BPC], F32)
        nc.scalar.activation(out=g_t, in_=g_u,
                             func=mybir.ActivationFunctionType.Tanh,
                             scale=0.7978845608028654)
        hT_sb = gpool.tile([128, DFT, BPC], F32)
        nc.vector.scalar_tensor_tensor(out=hT_sb.rearrange("p j b -> p (j b)"),
                                       in0=g_t, scalar=1.0, in1=g_xh,
                                       op0=AluOpType.add, op1=AluOpType.mult)

        # layer 2 (+ K=1 ones-row matmul folding gate_b2)
        psum_l = gps.tile([BPC, E], F32, bufs=2, tag="psl")
        for j in range(DFT):
            nc.tensor.matmul(out=psum_l, lhsT=hT_sb[:, j, :], rhs=w2_sb[:, j, :],
                             start=(j == 0), stop=False)
        nc.tensor.matmul(out=psum_l, lhsT=ones_sb[0:1, 0:BPC], rhs=b2_sb,
                         start=False, stop=True)
        logits_my = gpool.tile([BPC, E], F32)
        lev = nc.vector.tensor_copy(out=logits_my, in_=psum_l)

        # top-2 gates: g2 = sigmoid(l2 - l1), g1 = 1 - g2
        sorted8 = gpool.tile([BPC, E], F32)
        sidx = gpool.tile([BPC, E], U32)
        nc.vector.max(out=sorted8, in_=logits_my)
        nc.vector.max_index(out=sidx, in_max=sorted8, in_values=logits_my)
        diff = gpool.tile([BPC, 1], F32)
        nc.vector.tensor_tensor(out=diff, in0=sorted8[:, 1:2],
                                in1=sorted8[:, 0:1], op=AluOpType.subtract)
        g2c = gpool.tile([BPC, 1], F32)
        nc.scalar.activation(out=g2c, in_=diff,
                             func=mybir.ActivationFunctionType.Sigmoid,
                             scale=1.0)
        g1c = gpool.tile([BPC, 1], F32)
        nc.vector.tensor_scalar(g1c, g2c, -1.0, 1.0,
                                AluOpType.mult, AluOpType.add)

        # broadcast g1/g2 of each battery to all 128 partitions with one
        # matmul: rhs = [diag(g1) | diag(g2)] (8 x 16), lhsT = ones (8 x 128).
        rhs8 = gpool.tile([BPC, 2, BPC], F32)
        nc.vector.tensor_scalar_mul(rhs8[:, 0, :], id64[0:BPC, 0:BPC], g1c)
        nc.scalar.activation(out=rhs8[:, 1, :], in_=id64[0:BPC, 0:BPC],
                             func=mybir.ActivationFunctionType.Copy,
                             scale=g2c)
        psum_bc = gps.tile([128, 2 * BPC], F32, bufs=2, tag="pbc")
        nc.tensor.matmul(out=psum_bc, lhsT=ones_sb,
                         rhs=rhs8.rearrange("p s b -> p (s b)"),
                         start=True, stop=True)
        bcA = gpool.tile([128, 2, BPC], F32)
        nc.vector.tensor_copy(out=bcA.rearrange("p s b -> p (s b)"), in_=psum_bc)

        # PE warm-up gated on the first h8 eviction: keeps the HAM
        # clock-gate at full speed through the transpose/L2/top-2 chain
        # and into the fused main-loop matmuls.
        jps2 = gps.tile([BPC, 512], F32, bufs=2, tag="pbc")
        for j in range(NWARM):
            jmm = nc.tensor.matmul(
                out=jps2, lhsT=jt[:, 0:BPC], rhs=jt,
                start=True, stop=True,
            )
            if j == 0:
                add_dep_helper(jmm.ins, hev0.ins, sync=True,
                               reason="warm-up spans the post-L1 chain")

        gps_ctx.close()
        w1_ctx.close()

        # ---------- main fused phase -----------------------------------
        mps = ctx.enter_context(tc.tile_pool(name="mpsum", bufs=2, space="PSUM"))
        wbpool = ctx.enter_context(tc.tile_pool(name="wbs", bufs=2))
        scpool = ctx.enter_context(tc.tile_pool(name="scratch", bufs=2))
        opool = ctx.enter_context(tc.tile_pool(name="outs", bufs=3))

        def _vload(eng, ap, name):
            reg = eng.alloc_register(name)
            eng.reg_load(reg, ap)
            val = eng.snap(reg, donate=True)
            return nc.s_assert_within(val, 0, E - 1, skip_runtime_assert=True)

        def combine(b, pieces=2):
            """wb = g1*A_e1 + g2*A_e2 for battery b, 2 fused ops per piece."""
            rv1 = _vload(nc.vector, sidx[b:b + 1, 0:1], f"e1_{b}")
            rv2 = _vload(nc.scalar, sidx[b:b + 1, 1:2], f"e2_{b}")
            wb = wbpool.tile([128, KT, D], BF16)
            w = KT // pieces
            for h in range(pieces):
                kts = slice(h * w, (h + 1) * w)
                t2 = scpool.tile([128, w, D], BF16, tag=f"t2_{pieces}")
                nc.scalar.activation(
                    out=t2.rearrange("p k d -> p (k d)"),
                    in_=A_sb[:, ds(rv2, 1), kts, :].rearrange("p o k d -> p (o k d)"),
                    func=mybir.ActivationFunctionType.Copy,
                    scale=bcA[:, 1, b:b + 1],
                )
                nc.vector.scalar_tensor_tensor(
                    out=wb[:, kts, :].rearrange("p k d -> p (k d)"),
                    in0=A_sb[:, ds(rv1, 1), kts, :].rearrange("p o k d -> p (o k d)"),
                    scalar=bcA[:, 0, b:b + 1],
                    in1=t2.rearrange("p k d -> p (k d)"),
                    op0=AluOpType.mult, op1=AluOpType.add,
                )
            return wb

        def battery(b, wb):
            xb = xb_tiles[b]
            pm = mps.tile([128, MT, D], F32, tag="mp")
            for kt in range(KT):
                np_ = KREM if kt == 7 else 128
                for m in range(MT):
                    nc.tensor.matmul(
                        out=pm[:, m, :],
                        lhsT=xb[0:np_, kt, ts(m, 128)],
                        rhs=wb[0:np_, kt, :],
                        start=(kt == 0), stop=(kt == KT - 1),
                    )
            osb = opool.tile([128, MT, D], BF16, tag="osb")
            nc.vector.tensor_copy(
                out=osb[:, 0:2, :].rearrange("p m d -> p (m d)"),
                in_=pm[:, 0:2, :].rearrange("p m d -> p (m d)"),
            )
            nc.scalar.activation(
                out=osb[:, 2:4, :].rearrange("p m d -> p (m d)"),
                in_=pm[:, 2:4, :].rearrange("p m d -> p (m d)"),
                func=mybir.ActivationFunctionType.Copy,
            )
            return nc.sync.dma_start(
                out=outd.ap()[b].rearrange("p m d -> p (m d)"),
                in_=osb.rearrange("p m d -> p (m d)"),
            )

        wbs = {0: combine(0, pieces=4), 1: combine(1, pieces=4)}
        for b in range(BPC):
            battery(b, wbs.pop(b))
            if b + 2 < BPC:
                wbs[b + 2] = combine(b + 2)


def make_nc():
    from concourse import bacc
    nc = bacc.Bacc("TRN2", target_bir_lowering=False, debug=False,
                   num_devices=NCORES)
    build_program(nc)
    nc.finalize()
    return nc


def prep_inputs(cycle_curve_data, cycle_numbers, DKP_embeddings,
                gate_W1, gate_b1, gate_W2, gate_b2,
                expert_W, expert_b, gen_W, gen_b):
    """Host-side layout prep (reshape/pad/cast/weight-fold). Returns in_maps."""
    f32 = np.float32
    bf16 = ml_dtypes.bfloat16

    # fused expert weights A_e = gen_W + expert_W[e]; ones-row bias.
    A = np.empty((E, K, D), dtype=f32)
    A[:, :CF, :] = np.asarray(expert_W, dtype=f32) + np.asarray(gen_W, dtype=f32)
    A[:, CF, :] = np.asarray(expert_b, dtype=f32) + np.asarray(gen_b, dtype=f32)
    Abf = A.astype(bf16)
    # [128, 7(kt), E, D] so each k-tile is one contiguous DMA chunk.
    amain = np.ascontiguousarray(
        Abf[:, :896, :].reshape(E, 7, 128, D).transpose(2, 1, 0, 3))
    arem = np.ascontiguousarray(Abf[:, 896:K, :].transpose(1, 0, 2))

    # x transposed with ones-row, partition-major.
    x = np.asarray(cycle_curve_data, dtype=f32).reshape(B, L, CF)
    xT = np.empty((B, K, L), dtype=bf16)
    xT[:, :CF, :] = x.transpose(0, 2, 1).astype(bf16)
    xT[:, CF, :] = np.asarray(1.0, dtype=bf16)
    xmain = np.ascontiguousarray(
        xT[:, :896, :].reshape(B, 7, 128, L).transpose(0, 2, 1, 3))
    xrem = np.ascontiguousarray(xT[:, 896:K, :])

    # gating input, partition-major, per-core battery slice.
    g = np.zeros((GK, B), dtype=f32)
    g[:DLLM, :] = np.asarray(DKP_embeddings, dtype=f32).T
    g[DLLM, :] = np.asarray(cycle_numbers, dtype=f32)[:, 0]
    g[DLLM + 1, :] = 1.0
    gpm = g.reshape(GKT, 128, B).transpose(1, 0, 2).astype(np.float16)

    # full W1 (replicated to every core), partition-major.
    W1p = np.zeros((GK, DFF), dtype=f32)
    W1p[:DLLM + 1, :] = np.asarray(gate_W1, dtype=f32)
    W1p[DLLM + 1, :] = np.asarray(gate_b1, dtype=f32)
    w1f = np.ascontiguousarray(
        W1p.reshape(GKT, 128, DFF).transpose(1, 0, 2)
        .reshape(128, GKT * DFF).astype(np.float16))

    w2f = np.ascontiguousarray(
        np.asarray(gate_W2, dtype=f32).reshape(DFT, 128, E)
        .transpose(1, 0, 2).reshape(128, DFT * E))
    b2row = np.ascontiguousarray(np.asarray(gate_b2, dtype=f32).reshape(1, E))
    id64 = np.eye(B, dtype=f32)

    in_maps = []
    for c in range(NCORES):
        ginp = np.ascontiguousarray(
            gpm[:, :, c * BPC:(c + 1) * BPC].reshape(128, GKT * BPC))
        in_maps.append({
            "xmain": np.ascontiguousarray(xmain[c * BPC:(c + 1) * BPC]),
            "xrem": np.ascontiguousarray(xrem[c * BPC:(c + 1) * BPC]),
            "amain": amain,
            "arem": arem,
            "ginp": ginp,
            "w1f": w1f,
            "w2f": w2f,
            "b2row": b2row,
            "id64": id64,
        })
    return in_maps


_CACHED = {}


def run(inputs, trace=False, tmpdir=None):
    """Run on the 8 NeuronCores; returns (full_output, BassKernelResults)."""
    from concourse import bass_utils
    in_maps = prep_inputs(**inputs)
    nc = _CACHED.get("nc")
    if nc is None:
        nc = make_nc()
        _CACHED["nc"] = nc
    res = bass_utils.run_bass_kernel_spmd(
        nc, in_maps, core_ids=list(range(NCORES)), trace=trace, tmpdir=tmpdir
    )
    outs = [np.asarray(r["out"]) for r in res.results]
    full = np.concatenate(outs, axis=0)          # [B, 128, MT, D] bf16
    full = full.transpose(0, 2, 1, 3).reshape(B, L, D).astype(np.float32)
    return full, res


def kernel(**inputs):
    full, _ = run(inputs, trace=False)
    return full
